# revision 19
# baseline (speedup 1.0000x reference)
"""ABC attention (gated slot attention) on 8 TRN2 NeuronCores.

Sharding: 2 heads per core (16 heads / 8 cores). Per core:
  - projections q,k (RoPE, q pre-scaled), v, silu(gate), slot logits,
    all matmuls bf16 (2x stream rate vs f32r), moving dim 512
  - unnormalized softmax: RMS-norm downstream is scale-invariant, so
    softmax keeps only exp(ok*enz)*enz; enz applied in [m,t] layout
    (enzT) -> no per-row transposes/reductions/reciprocals
  - LINEAR-ATTENTION restructure: the unmasked (off-diagonal) prefix of
    the two quadratic stages collapses into rank-64 running states
      B[dk,m]  = sum_{j<prefix} k[j,dk] es[j,m]   (stage 1)
      CT[m,dv] = sum_{j<prefix} es[j,m] v[j,dv]   (stage 2)
    accumulated in a persistent PSUM bank; only the diagonal 512x512
    superblock runs the masked quadratic path (with per-128-block
    moving-dim shrink). Exact reassociation of the same unnormalized
    sums the quadratic version computes.
  - fused RMS-norm x gate epilogue (Rsqrt broadcast via PE)
  - AllToAll reshards o_g head-split -> T-split (1MB/core vs 8.4MB
    AllGather); full-size warm-up A2A mid-attention; A2A(h0) issued
    before h1's epilogue; o_proj accumulates h0 chunks first so it
    overlaps A2A(h1); PE filler matmuls hold DVFS clock during A2A(h0)
    and during the startup DMA window.
"""
import sys
if '/opt/trn_rl_repo' not in sys.path:
    sys.path.insert(0, '/opt/trn_rl_repo')
import numpy as np

import concourse.bacc as bacc
import concourse.mybir as mybir
import concourse.tile as tile
from concourse import bass_utils

F32 = mybir.dt.float32
F32R = mybir.dt.float32r
BF16 = mybir.dt.bfloat16
AF = mybir.ActivationFunctionType

H, DK, DV, M, T, D = 16, 128, 128, 64, 2048, 2048
EPS, CLAMP, ROPE_BASE = 1e-5, 32.0, 10000.0
N_CORES = 8
NT = T // 128        # 16
NB = T // 512        # 4 big row-chunks
ND = D // 128        # 16
SCALE = DK ** -0.5

REPEAT = 1
DEBUG = False
STARTUP_FILLERS = 44
A2A_FILLERS = 95


def build(repeat=1, debug=False):
    nc = bacc.Bacc(None, target_bir_lowering=False, debug=False, num_devices=N_CORES)

    din = {}
    for nm, shp, dt in [
        ("hsb", [128, NB, ND, 512], BF16),
        ("wq0", [128, ND, 128], BF16), ("wq1", [128, ND, 128], BF16),
        ("wk0", [128, ND, 128], BF16), ("wk1", [128, ND, 128], BF16),
        ("wg0", [128, ND, 128], BF16), ("wg1", [128, ND, 128], BF16),
        ("wv0", [128, ND, 128], BF16), ("wv1", [128, ND, 128], BF16),
        ("ws1", [128, ND, 16], BF16),
        ("ws2e", [17, 128], BF16),
        ("ones2k", [1, T], BF16),
        ("onesrow_b", [1, 128], BF16), ("onescol_b", [128, 1], BF16),
        ("onesrow_r", [1, 128], F32R), ("onescol_r", [128, 1], F32R),
        ("cossin", [128, T], F32),
        ("triu", [128, 128], BF16), ("ident", [128, 128], F32),
        ("identb", [128, 128], BF16),
        ("masks", [128, 4, 512], BF16),
        ("woT", [128, ND, D], BF16),
    ]:
        din[nm] = nc.dram_tensor(nm, shp, dt, kind="ExternalInput")
    out_d = nc.dram_tensor("out", [256, D], BF16, kind="ExternalOutput")

    dbg = {}
    if debug:
        for nm, shp, dt in [("qT", [256, T], BF16), ("kT", [256, T], BF16),
                            ("v", [128, NT * 256], BF16),
                            ("sg", [256, T], BF16), ("es", [128, NT * 128], BF16),
                            ("enzT", [128, T], F32), ("esT", [128, T], BF16),
                            ("u17", [17, T], BF16), ("qveT", [128, T], BF16),
                            ("ogT", [256, T], BF16)]:
            dbg[nm] = nc.dram_tensor("dbg_" + nm, shp, dt, kind="ExternalOutput")

    with tile.TileContext(nc) as tc:
        with tc.tile_pool(name="const", bufs=1) as cpool, \
             tc.tile_pool(name="big", bufs=1) as big:
            c = {}
            for nm in ("ws2e", "onesrow_b", "onescol_b", "onesrow_r",
                       "onescol_r", "triu", "ident", "identb"):
                tl = cpool.tile(list(din[nm].shape), din[nm].dtype, tag=nm, name=nm)
                nc.sync.dma_start(tl[:], din[nm].ap())
                c[nm] = tl

            st = {
                "bigpool": big,
                "qT": [big.tile([128, T], BF16, tag=f"qT{h}", name=f"qT{h}") for h in range(2)],
                "kT": [big.tile([128, T], BF16, tag=f"kT{h}", name=f"kT{h}") for h in range(2)],
                "k_t": [big.tile([128, NT, 128], BF16, tag=f"k_t{h}", name=f"k_t{h}") for h in range(2)],
                "sg": [big.tile([128, T], BF16, tag=f"sg{h}", name=f"sg{h}") for h in range(2)],
                "v": big.tile([128, NT, 256], BF16, tag="v", name="v"),
                "u17": big.tile([17, T], BF16, tag="u17", name="u17"),
                "es_t": big.tile([128, NT, 128], BF16, tag="es_t", name="es_t"),
                "esT": big.tile([128, T], BF16, tag="esT", name="esT"),
                "enzT": big.tile([128, T], F32, tag="enzT", name="enzT"),
                "B_sb": big.tile([128, 2, 64], BF16, tag="B_sb", name="B_sb"),
                "CT_sb": big.tile([128, 128], BF16, tag="CT_sb", name="CT_sb"),
            }
            for _ in range(repeat):
                _pass(nc, tc, din, c, st, out_d, dbg, debug)

    nc.compile()
    return nc, dbg


def _pass(nc, tc, din, c, st, out_d, dbg, debug):
    qT, kT, sg = st["qT"], st["kT"], st["sg"]
    v, u17, es_t, esT, enzT = st["v"], st["u17"], st["es_t"], st["esT"], st["enzT"]
    k_t, B_sb, CT_sb = st["k_t"], st["B_sb"], st["CT_sb"]

    def cpy(eng, dst, srcap):
        if eng is nc.scalar:
            nc.scalar.copy(dst, srcap)
        elif eng is nc.gpsimd:
            nc.gpsimd.tensor_copy(dst, srcap)
        else:
            nc.vector.tensor_copy(dst, srcap)

    # ================= PHASE 1: projections (4 x 512-col sweeps) =================
    with tc.tile_pool(name="p1w", bufs=1) as p1w, \
         tc.tile_pool(name="p1sb", bufs=2) as p1sb, \
         tc.tile_pool(name="p1hs", bufs=4) as p1hs:
        p1ps_cm = tc.tile_pool(name="p1ps", bufs=1, space="PSUM")
        p1ps = p1ps_cm.__enter__()
        # hs chunk 0 first (4 sub-tiles of 4 d-groups each), then weights:
        # the first matmul needs only hsq sub 0 + wq0, so compute starts
        # ~6us in instead of waiting for the full weight set
        engs = [nc.sync, nc.scalar, nc.gpsimd]

        # PE fillers: a memset tile needs no DMA, so the PE ramps its DVFS
        # clock during the initial weight/hs DMA window instead of idling
        warm_sb = p1sb.tile([128, 640], BF16, tag="warm_sb", bufs=1)
        nc.vector.memset(warm_sb[:], 0.0)
        fill_ps = p1ps.tile([128, 512], F32, tag="pa0", name="fill_ps")
        for i in range(STARTUP_FILLERS):
            nc.tensor.matmul(fill_ps[:], warm_sb[:, 0:128], warm_sb[:, 128:640],
                             start=True, stop=True)

        # weights on the scalar queue (clear of the 4MB hs stream on
        # sync/gpsimd) so the first matmul group unblocks ~5us in
        nc.gpsimd.dma_start(u17[16:17, :], din["ones2k"].ap())

        def hs_load_pair(tb0, tb1):
            # interleave the two chunks' sub-DMAs so sub0 of BOTH chunks
            # lands first (the d-loop consumes ti=0/ti=1 per d)
            subs = [[], []]
            for s in range(4):
                for ti, tb in ((0, tb0), (1, tb1)):
                    t = p1hs.tile([128, 4, 512], BF16, tag=f"hsq{s}",
                                  name=f"hsq{tb}_{s}")
                    eng = nc.sync if s % 2 == 0 else nc.gpsimd
                    eng.dma_start(t[:], din["hsb"].ap()[:, tb, 4 * s:4 * s + 4])
                    subs[ti].append(t)
            return subs

        hs_next = hs_load_pair(0, 1)
        wts = {}
        weng = {"wq0": nc.scalar, "wq1": nc.scalar, "wk0": nc.scalar,
                "wk1": nc.scalar, "wg0": nc.scalar, "wg1": nc.scalar,
                "wv0": nc.sync, "wv1": nc.sync, "ws1": nc.scalar}
        for wn in ("wq0", "wq1", "wk0", "wk1", "wg0", "wg1", "wv0", "wv1",
                   "ws1"):
            cw = din[wn].shape[2]
            wt = p1w.tile([128, ND, cw], BF16, tag=wn, name=wn)
            weng[wn].dma_start(wt[:], din[wn].ap())
            wts[wn] = wt
        cossin = p1w.tile([128, T], F32, tag="cossin", name="cossin")
        nc.scalar.dma_start(cossin[:], din["cossin"].ap())

        carries = []

        def mk_pt(pool, tags):
            state = {"i": 0}

            def pt(shape, name):
                t = pool.tile(shape, F32, tag=tags[state["i"] % len(tags)],
                              name=name)
                state["i"] += 1
                return t
            return pt

        def slot_chain(tb, pt, sbp):
            # staged: pse/esT -> pssq/es_t (one wide exp) -> csum chain ->
            # cs2 quad -> one wide recip -> transpose quad -> one enzT copy
            tsl = slice(tb * 512, (tb + 1) * 512)
            t0 = 4 * tb
            pse = pt([128, 512], f"psesT{tb}")
            nc.tensor.matmul(pse[:], c["ws2e"][:], u17[:, tsl], start=True, stop=True)
            nc.scalar.activation(esT[:, tsl], pse[:], AF.Exp)
            pssq = pt([128, 4, 128], f"ps_st{tb}")
            for i in range(4):
                nc.tensor.matmul(pssq[:, i, :],
                                 u17[:, (t0 + i) * 128:(t0 + i + 1) * 128],
                                 c["ws2e"][:], start=True, stop=True)
            nc.scalar.activation(
                es_t[:, t0:t0 + 4, :].rearrange("p a b -> p (a b)"),
                pssq[:].rearrange("p a b -> p (a b)"), AF.Exp)
            for i in range(4):
                ts = t0 + i
                if ts >= NT - 1:
                    break
                csum = pt([1, 128], f"csum{ts}")
                nc.tensor.matmul(csum[:], c["onescol_b"][:], es_t[:, ts, :],
                                 start=True, stop=(ts == 0))
                if ts > 0:
                    nc.tensor.matmul(csum[:], c["onesrow_b"][:, 0:1],
                                     carries[ts - 1][:], start=False, stop=True)
                cr = st["bigpool"].tile([1, 128], BF16, tag=f"carry{ts}",
                                        name=f"carry{ts}")
                nc.vector.tensor_copy(cr[:], csum[:])
                carries.append(cr)
            cs2q = pt([128, 4, 128], f"ps_cs{tb}")
            for i in range(4):
                ts = t0 + i
                nc.tensor.matmul(cs2q[:, i, :], c["triu"][:], es_t[:, ts, :],
                                 start=True, stop=(ts == 0))
                if ts > 0:
                    nc.tensor.matmul(cs2q[:, i, :], c["onesrow_b"][:],
                                     carries[ts - 1][:], start=False, stop=True)
            enz_sb = sbp.tile([128, 512], F32, tag="enz_sb", name=f"enz{tb}")
            nc.vector.reciprocal_approx_fast(
                enz_sb[:], cs2q[:].rearrange("p a b -> p (a b)"))
            etpq = pt([128, 4, 128], f"etp{tb}")
            for i in range(4):
                nc.tensor.transpose(etpq[:, i, :],
                                    enz_sb[:, i * 128:(i + 1) * 128],
                                    c["ident"][:])
            nc.scalar.copy(enzT[:, tsl], etpq[:].rearrange("p a b -> p (a b)"))
        st["slot_chain"] = slot_chain
        st["mk_pt"] = mk_pt

        QKG = {"wq0": (qT[0], "q"), "wq1": (qT[1], "q"),
               "wk0": (kT[0], "k"), "wk1": (kT[1], "k"),
               "wg0": (sg[0], "g"), "wg1": (sg[1], "g")}
        GROUPS = (("wq0", "wq1"), ("wk0", "wk1"), ("wg0", "wg1"))
        TAGSETS = (("pa0", "pa1", "pa2", "pa3"), ("pb0", "pb1", "pb2", "pb3"))

        def drain_qkg(wn, ti, ps):
            # drains: RoPE for q/k (q pre-scaled), silu for g
            dst, kind = QKG[wn]
            tsl = slice(ti * 512, (ti + 1) * 512)
            if kind == "g":
                nc.scalar.activation(dst[:, tsl], ps[:], AF.Silu)
            else:
                t1 = p1sb.tile([64, 512], F32, tag="ropet1")
                t2 = p1sb.tile([64, 512], F32, tag="ropet2")
                nc.vector.tensor_mul(t1[:], ps[0:64, :], cossin[0:64, tsl])
                nc.vector.tensor_mul(t2[:], ps[64:128, :], cossin[64:128, tsl])
                nc.vector.tensor_sub(dst[0:64, tsl], t1[:], t2[:])
                nc.vector.tensor_mul(t1[:], ps[64:128, :], cossin[0:64, tsl])
                nc.vector.tensor_mul(t2[:], ps[0:64, :], cossin[64:128, tsl])
                nc.vector.tensor_add(dst[64:128, tsl], t1[:], t2[:])

        # paired 512-chunks: each weight tile loads once per PAIR of chunks
        # (consecutive same-stationary matmuls run at stream rate)
        hs_pairs = []
        for pair in range(2):
            tbs = (2 * pair, 2 * pair + 1)
            hsq_p = hs_next
            if pair == 0:
                hs_next = hs_load_pair(2, 3)

            def hs_d(ti, d):
                return hsq_p[ti][d // 4][:, d % 4, :]

            for gi, grp in enumerate(GROUPS):
                tags = TAGSETS[(2 * pair + gi) % 2]
                acc = {}
                for wi, wn in enumerate(grp):
                    for ti in range(2):
                        acc[(wn, ti)] = p1ps.tile(
                            [128, 512], F32, tag=tags[2 * wi + ti],
                            name=f"acc_{wn}_{ti}_p{pair}")
                for d in range(ND):
                    for wn in grp:
                        for ti in range(2):
                            nc.tensor.matmul(acc[(wn, ti)][:], wts[wn][:, d, :],
                                             hs_d(ti, d),
                                             start=(d == 0), stop=(d == ND - 1))
                for wn in grp:
                    for ti in range(2):
                        drain_qkg(wn, tbs[ti], acc[(wn, ti)])
            hs_pairs.append(hsq_p)

        # ---- v pass: dk-major vT with 4-chunk same-stationary chains ----
        # (stationary = wv block, fixed across all 4 t-chunks -> 2048
        # moving cols per LDWEIGHTS instead of 272)
        def hs_all(ci, d):
            return hs_pairs[ci // 2][ci % 2][d // 4][:, d % 4, :]

        vacc = {}
        for b in range(2):
            for ci in range(4):
                vacc[(b, ci)] = p1ps.tile(
                    [128, 512], F32, tag=TAGSETS[b // 1][ci] if False else
                    ("pa", "pb")[b] + str(ci), name=f"vacc{b}{ci}")
        for d in range(ND):
            for b in range(2):
                for ci in range(4):
                    nc.tensor.matmul(vacc[(b, ci)][:], wts[f"wv{b}"][:, d, :],
                                     hs_all(ci, d),
                                     start=(d == 0), stop=(d == ND - 1))
        # transpose vT -> t-major v through the PE
        for b in range(2):
            for ci in range(4):
                vt_sb = p1sb.tile([128, 512], BF16, tag="vt_sb")
                cpy(nc.scalar if (b + ci) % 2 == 0 else nc.vector,
                    vt_sb[:], vacc[(b, ci)][:])
                trp = p1ps.tile([128, 4, 128], BF16, tag=("pa", "pb")[b] + str(ci),
                                name=f"vtr{b}{ci}")
                for q in range(4):
                    nc.tensor.transpose(trp[:, q, :],
                                        vt_sb[:, q * 128:(q + 1) * 128],
                                        c["identb"][:])
                cpy(nc.vector if (b + ci) % 2 == 0 else nc.scalar,
                    v[:, 4 * ci:4 * ci + 4, b * 128:(b + 1) * 128],
                    trp[:])
        # ---- u pass: uT = Ws1 hs directly in [16, t] layout ----
        uacc = [p1ps.tile([16, 512], F32, tag="pa" + str(ci), name=f"uacc{ci}")
                for ci in range(4)]
        for d in range(ND):
            for ci in range(4):
                nc.tensor.matmul(uacc[ci][:], wts["ws1"][:, d, :],
                                 hs_all(ci, d),
                                 start=(d == 0), stop=(d == ND - 1))
        for ci in range(4):
            cpy(nc.scalar if ci % 2 == 0 else nc.vector,
                u17[0:16, ci * 512:(ci + 1) * 512], uacc[ci][:])
        # slot chains 0,1 here (need u17); 2,3 deferred to phase 2
        for tb in range(2):
            slot_chain(tb, mk_pt(p1ps, ("pb2", "pb3")), p1sb)
        p1ps_cm.__exit__(None, None, None)

    if debug:
        for h in range(2):
            nc.sync.dma_start(dbg["qT"].ap()[h * 128:(h + 1) * 128, :], qT[h][:])
            nc.sync.dma_start(dbg["kT"].ap()[h * 128:(h + 1) * 128, :], kT[h][:])
            nc.sync.dma_start(dbg["sg"].ap()[h * 128:(h + 1) * 128, :], sg[h][:])
        nc.sync.dma_start(dbg["v"].ap(), v[:].rearrange("p a b -> p (a b)"))
        nc.sync.dma_start(dbg["es"].ap(), es_t[:].rearrange("p a b -> p (a b)"))
        nc.sync.dma_start(dbg["enzT"].ap(), enzT[:])
        nc.sync.dma_start(dbg["esT"].ap(), esT[:])
        nc.sync.dma_start(dbg["u17"].ap(), u17[:])

    # ================= PHASE 2: attention =================
    with tc.tile_pool(name="p2sb", bufs=3) as p2sb, \
         tc.tile_pool(name="qveP", bufs=2) as qvep, \
         tc.tile_pool(name="dramp", bufs=1, space="DRAM") as dpool:
        ag_in, ag_out = [], []
        for h in range(2):
            ag_in.append(dpool.tile([N_CORES, 128, 256], BF16,
                                    tag=f"ag_in{h}", name=f"ag_in{h}"))
            ag_out.append(dpool.tile([N_CORES, 128, 256], BF16,
                                     tag=f"ag_out{h}", name=f"ag_out{h}"))
        warm_in = dpool.tile([N_CORES, 128, 256], BF16, tag="warm_in", name="warm_in")
        warm_out = dpool.tile([N_CORES, 128, 256], BF16, tag="warm_out", name="warm_out")
        wsrc = p2sb.tile([128, 256], BF16, tag="wsrc", bufs=1)
        nc.vector.memset(wsrc[:], 0.0)
        for s in range(N_CORES):
            nc.sync.dma_start(warm_in[s, :, :], wsrc[:])
        epsb = p2sb.tile([128, 1], F32, tag="epsb", bufs=1)
        nc.vector.memset(epsb[:], EPS)
        fsrc = p2sb.tile([128, 640], BF16, tag="fsrc", bufs=1)
        nc.vector.memset(fsrc[:], 0.0)
        tl = p2sb.tile(list(din["masks"].shape), BF16, tag="masks", name="masks", bufs=1)
        nc.sync.dma_start(tl[:], din["masks"].ap())
        c["masks"] = tl
        # o_proj weights: prefetch the full 8MB during attention
        wo_sb = p2sb.tile([128, ND, D], BF16, tag="wo_sb", name="wo_sb", bufs=1)
        nc.sync.dma_start(wo_sb[:], din["woT"].ap())

        with tc.tile_pool(name="ps_atw", bufs=1, space="PSUM") as ps_at, \
             tc.tile_pool(name="ps_ok", bufs=1, space="PSUM") as ps_ok, \
             tc.tile_pool(name="ps_ot", bufs=2, space="PSUM") as ps_ot, \
             tc.tile_pool(name="ps_pre", bufs=1, space="PSUM") as ps_pre:
            # PSUM: at(2) + w2(2, also epilogue aux) + okT(1) + oT(2) + pre(1)
            # persistent rank-64 prefix states, packed into one PSUM bank:
            #   B[h]  = pre[:, h*64:(h+1)*64]            [dk=128, m=64]
            #   CT[h] = pre[h*64:(h+1)*64, 128:256]      [m=64, dv=128]
            pre = ps_pre.tile([128, 512], F32, tag="pre", name="pre")

            def B_ps(h):
                return pre[:, h * 64:(h + 1) * 64]

            def CT_ps(h):
                return pre[h * 64:(h + 1) * 64, 128:256]

            def mk_pt2(pool):
                def pt(shape, name):
                    return pool.tile(shape, F32, tag="oT", name=name)
                return pt

            def gen_pre():
                # k in t-major layout for the B prefix updates: transpose kT
                # 128-blocks through the PE (4 at a time into one PSUM tile);
                # interleaved with stage1(0) as independent PE work
                for h in range(2):
                    for qd in range(4):
                        trp = ps_ot.tile([128, 4, 128], BF16, tag="oT",
                                         name=f"ktr{h}{qd}")
                        for i in range(4):
                            ts = 4 * qd + i
                            nc.tensor.transpose(trp[:, i, :],
                                                kT[h][:, ts * 128:(ts + 1) * 128],
                                                c["identb"][:])
                        eng = nc.scalar if (h + qd) % 2 == 0 else nc.vector
                        cpy(eng,
                            k_t[h][:, 4 * qd:4 * qd + 4, :].rearrange("p a b -> p (a b)"),
                            trp[:].rearrange("p a b -> p (a b)"))
                        yield
                st["slot_chain"](NB - 2, mk_pt2(ps_ot), p2sb)
                yield
                st["slot_chain"](NB - 1, mk_pt2(ps_ot), p2sb)
                yield

            def softmax(I, okT):
                # unnormalized: qveT = exp(okT * enzT) * enzT  (RMS-norm
                # downstream cancels the per-(t,h) softmax denominator)
                rsl = slice(I * 512, (I + 1) * 512)
                qveT = qvep.tile([128, 512], BF16, tag="qveT")
                okm = p2sb.tile([128, 512], F32, tag="okm")
                nc.vector.tensor_mul(okm[:], okT[:], enzT[:, rsl])
                eok = p2sb.tile([128, 512], F32, tag="eok")
                nc.scalar.activation(eok[:], okm[:], AF.Exp)
                nc.vector.tensor_mul(qveT[:], eok[:], enzT[:, rsl])
                return qveT

            def stage1_steps(I, res):
                njs0 = 4 * I
                okT = ps_ok.tile([128, 512], F32, tag="okT", name=f"okT{I}")
                at_sb = {}

                def at_pair(m):
                    j = njs0 + m
                    lo = 128 * m if I > 0 else 0
                    for h in range(2):
                        at = ps_at.tile([128, 512], F32, tag="at", bufs=2)
                        nc.tensor.matmul(at[:, lo:512],
                                         kT[h][:, j * 128:(j + 1) * 128],
                                         qT[h][:, I * 512 + lo:(I + 1) * 512],
                                         start=True, stop=True)
                        a_sb = p2sb.tile([128, 512], BF16, tag="at_sb")
                        if I == 0:
                            nc.vector.tensor_mul(a_sb[:], at[:], c["masks"][:, m, :])
                        else:
                            nc.vector.tensor_mul(
                                a_sb[:, lo:lo + 128], at[:, lo:lo + 128],
                                c["masks"][:, m, lo:lo + 128])
                            if lo + 128 < 512:
                                eng = nc.scalar if h == 0 else nc.vector
                                cpy(eng, a_sb[:, lo + 128:512],
                                    at[:, lo + 128:512])
                        at_sb[(m, h)] = (a_sb, lo)

                def ok_mm(m):
                    for h in range(2):
                        a_sb, lo = at_sb.pop((m, h))
                        nc.tensor.matmul(okT[h * 64:(h + 1) * 64, lo:512],
                                         es_t[:, njs0 + m, h * 64:(h + 1) * 64],
                                         a_sb[:, lo:512],
                                         start=(I == 0 and m == 0),
                                         stop=(m == 3),
                                         skip_group_check=True)

                # off-diagonal prefix: okT[h] = B^T q  (rank-64 state)
                if I > 0:
                    for h in range(2):
                        nc.tensor.matmul(okT[h * 64:(h + 1) * 64, :],
                                         B_sb[:, h, :], qT[h][:, I * 512:(I + 1) * 512],
                                         start=True, stop=False,
                                         skip_group_check=True)
                at_pair(0)
                yield
                at_pair(1)
                yield
                for m in range(4):
                    ok_mm(m)
                    if m + 2 < 4:
                        at_pair(m + 2)
                    yield
                # softmax as early as possible: stage2(I) consumes qveT first
                # thing next iteration
                res["qv"] = softmax(I, okT)
                yield
                # B prefix update with this superblock's own k/es, then
                # refresh the bf16 stationary copy for stage1(I+1)
                if I < NB - 1:
                    for h in range(2):
                        for mi in range(4):
                            ts = njs0 + mi
                            nc.tensor.matmul(B_ps(h), k_t[h][:, ts, :],
                                             es_t[:, ts, h * 64:(h + 1) * 64],
                                             start=(I == 0 and mi == 0),
                                             stop=(mi == 3),
                                             skip_group_check=True)
                    yield
                    for h in range(2):
                        eng = nc.scalar if h == 0 else nc.vector
                        cpy(eng, B_sb[:, h, :], B_ps(h))
                if I == 2:
                    # full-size warm-up A2A: brings the collective path to
                    # steady-state so the real transfers at the end run warm
                    nc.gpsimd.collective_compute(
                        "AllToAll", mybir.AluOpType.bypass,
                        replica_groups=[list(range(N_CORES))],
                        ins=[warm_in[:].opt()], outs=[warm_out[:].opt()])
                yield

            def stage2_steps(I, qveT):
                rsl = slice(I * 512, (I + 1) * 512)
                njs0 = 4 * I
                oT = [ps_ot.tile([128, 512], F32, tag="oT", name=f"oT{i}") for i in range(2)]
                w2_sb = {}

                def w2_pair(m, hs=(0, 1)):
                    j = njs0 + m
                    lo = 128 * m if I > 0 else 0
                    for h in hs:
                        w2 = ps_at.tile([128, 512], F32, tag="w2", bufs=2)
                        nc.tensor.matmul(w2[:, lo:512],
                                         esT[h * 64:(h + 1) * 64, j * 128:(j + 1) * 128],
                                         qveT[h * 64:(h + 1) * 64, lo:512],
                                         start=True, stop=True)
                        wsb = p2sb.tile([128, 512], BF16, tag="at_sb")
                        if I == 0:
                            nc.vector.tensor_mul(wsb[:], w2[:], c["masks"][:, m, :])
                        else:
                            nc.vector.tensor_mul(
                                wsb[:, lo:lo + 128], w2[:, lo:lo + 128],
                                c["masks"][:, m, lo:lo + 128])
                            if lo + 128 < 512:
                                eng = nc.scalar if h == 0 else nc.vector
                                cpy(eng, wsb[:, lo + 128:512],
                                    w2[:, lo + 128:512])
                        w2_sb[(m, h)] = (wsb, lo)

                def o_mm(m, hs=(0, 1)):
                    for h in hs:
                        wsb, lo = w2_sb.pop((m, h))
                        nc.tensor.matmul(oT[h][:, lo:512],
                                         v[:, njs0 + m, h * 128:(h + 1) * 128],
                                         wsb[:, lo:512],
                                         start=(I == 0 and m == 0),
                                         stop=(m == 3),
                                         skip_group_check=True)

                def o_off(h):
                    nc.tensor.matmul(oT[h][:],
                                     CT_sb[h * 64:(h + 1) * 64, :],
                                     qveT[h * 64:(h + 1) * 64, :],
                                     start=True, stop=False,
                                     skip_group_check=True)

                def epilogue(h):
                    sq = p2sb.tile([128, 512], F32R, tag="sq")
                    nc.scalar.activation(sq[:], oT[h][:], AF.Square)
                    ssq = ps_at.tile([1, 512], F32, tag="w2", bufs=2)
                    nc.tensor.matmul(ssq[:], c["onescol_r"][:], sq[:], start=True, stop=True)
                    ssq_sb = p2sb.tile([1, 512], F32R, tag="ssq_sb")
                    with nc.allow_low_precision(reason="f32r bitcast for broadcast"):
                        nc.scalar.copy(ssq_sb[:], ssq[:])
                    rb = ps_at.tile([128, 512], F32, tag="w2", bufs=2)
                    nc.tensor.matmul(rb[:], c["onesrow_r"][:], ssq_sb[:], start=True, stop=True)
                    yield
                    rms = p2sb.tile([128, 512], F32, tag="rms")
                    nc.scalar.activation(rms[:], rb[:], AF.Sqrt, scale=1.0 / DV,
                                         bias=epsb[:])
                    rinv = p2sb.tile([128, 512], F32, tag="rinv")
                    nc.vector.reciprocal_approx_fast(rinv[:], rms[:])
                    t1 = p2sb.tile([128, 512], F32, tag="ept1")
                    nc.vector.tensor_mul(t1[:], oT[h][:], sg[h][:, rsl])
                    ogt = p2sb.tile([128, 512], BF16, tag="ogt")
                    nc.vector.tensor_mul(ogt[:], t1[:], rinv[:])
                    if debug:
                        nc.gpsimd.dma_start(dbg["ogT"].ap()[h * 128:(h + 1) * 128, rsl], ogt[:])
                    nc.sync.dma_start(ag_in[h][2 * I, :, :], ogt[:, 0:256])
                    nc.sync.dma_start(ag_in[h][2 * I + 1, :, :], ogt[:, 256:512])
                    if I == NB - 1:
                        # kick this head's A2A the moment its last slots land;
                        # h0's transfer overlaps h1's compute + o_proj(h0)
                        nc.gpsimd.collective_compute(
                            "AllToAll", mybir.AluOpType.bypass,
                            replica_groups=[list(range(N_CORES))],
                            ins=[ag_in[h][:].opt()], outs=[ag_out[h][:].opt()])
                    yield

                if I == NB - 1:
                    # last chunk: finish h0's whole path (accumulate, epilogue,
                    # A2A) before touching h1, so A2A(h0) fires ~8us earlier
                    # and rides a freshly-warmed collective path
                    nc.gpsimd.collective_compute(
                        "AllToAll", mybir.AluOpType.bypass,
                        replica_groups=[list(range(N_CORES))],
                        ins=[warm_in[:].opt()], outs=[warm_out[:].opt()])
                    for h in range(2):
                        o_off(h)
                        w2_pair(0, hs=(h,))
                        yield
                        w2_pair(1, hs=(h,))
                        yield
                        for m in range(4):
                            o_mm(m, hs=(h,))
                            if m + 2 < 4:
                                w2_pair(m + 2, hs=(h,))
                            yield
                        yield from epilogue(h)
                    return
                # off-diagonal prefix: oT[h] = CT^T qve  (rank-64 state)
                if I > 0:
                    for h in range(2):
                        o_off(h)
                w2_pair(0)
                yield
                w2_pair(1)
                yield
                for m in range(4):
                    o_mm(m)
                    if m + 2 < 4:
                        w2_pair(m + 2)
                    yield
                # CT prefix update + bf16 refresh
                if I < NB - 1:
                    for h in range(2):
                        for mi in range(4):
                            ts = njs0 + mi
                            nc.tensor.matmul(CT_ps(h),
                                             es_t[:, ts, h * 64:(h + 1) * 64],
                                             v[:, ts, h * 128:(h + 1) * 128],
                                             start=(I == 0 and mi == 0),
                                             stop=(mi == 3),
                                             skip_group_check=True)
                    yield
                    for h in range(2):
                        eng = nc.scalar if h == 0 else nc.vector
                        cpy(eng, CT_sb[h * 64:(h + 1) * 64, :], CT_ps(h))
                    yield
                # epilogue: o_g = o * rsqrt(mean o^2 + eps) * sg -> bf16 -> a2a_in
                for h in range(2):
                    yield from epilogue(h)

            def interleave(*gs):
                gens = list(gs)
                while gens:
                    for g in list(gens):
                        try:
                            next(g)
                        except StopIteration:
                            gens.remove(g)

            # software pipeline: stage1(I) and stage2(I-1) emit interleaved so
            # the PE queue always holds ready work from the other stream while
            # one stream waits on its vector/scalar producers
            res = {}
            interleave(stage1_steps(0, res), gen_pre())
            qv_prev = res["qv"]
            if debug:
                nc.sync.dma_start(dbg["qveT"].ap()[:, 0:512], qv_prev[:])
            for I in range(1, NB):
                interleave(stage1_steps(I, res), stage2_steps(I - 1, qv_prev))
                qv_prev = res["qv"]
                if debug:
                    nc.sync.dma_start(dbg["qveT"].ap()[:, I * 512:(I + 1) * 512], qv_prev[:])
            def filler_gen(n, per=6):
                # zero-data matmuls: keep the PE clock ramped through the
                # final stage2 + A2A window without burning switching power
                i = 0
                while i < n:
                    fps = ps_ok.tile([128, 512], F32, tag="okT", name=f"fill{i}")
                    for _ in range(min(per, n - i)):
                        nc.tensor.matmul(fps[:], fsrc[:, 0:128],
                                         fsrc[:, 128:640], start=True, stop=True)
                    i += per
                    yield

            interleave(stage2_steps(NB - 1, qv_prev), filler_gen(A2A_FILLERS))

        # ================= PHASE 3: per-head A2A + o_proj =================
        og = {}
        for h in range(2):
            for s in range(N_CORES):
                ot = p2sb.tile([128, 256], BF16, tag=f"og{h}{s}",
                               name=f"og{h}{s}", bufs=1)
                nc.sync.dma_start(ot[:], ag_out[h][s, :, :])
                og[(h, s)] = ot
        p3ps_cm = tc.tile_pool(name="p3ps", bufs=1, space="PSUM")
        p3ps = p3ps_cm.__enter__()
        pso = [[p3ps.tile([128, 512], F32, tag=f"pso{th}{ns}",
                          name=f"pso{th}{ns}", bufs=1) for ns in range(4)]
               for th in range(2)]
        # o_proj: h0 chunks first so the h0 half overlaps A2A(h1)
        order = [(0, s) for s in range(N_CORES)] + [(1, s) for s in range(N_CORES)]
        for n, (h, s) in enumerate(order):
            kc = 2 * s + h
            for th in range(2):
                for ns in range(4):
                    nc.tensor.matmul(pso[th][ns][:],
                                     og[(h, s)][:, th * 128:(th + 1) * 128],
                                     wo_sb[:, kc, ns * 512:(ns + 1) * 512],
                                     start=(n == 0), stop=(n == 15))
        dma_engs = [nc.sync, nc.scalar, nc.gpsimd]
        for th in range(2):
            for ns in range(4):
                osb = p2sb.tile([128, 512], BF16, tag="osb")
                eng = nc.scalar if ns % 2 == 0 else nc.vector
                cpy(eng, osb[:], pso[th][ns][:])
                dma_engs[(4 * th + ns) % 3].dma_start(
                    out_d.ap()[th * 128:(th + 1) * 128, ns * 512:(ns + 1) * 512],
                    osb[:])
        p3ps_cm.__exit__(None, None, None)


# ======================= host side =======================

def _host_inputs(inputs):
    import ml_dtypes
    BF = ml_dtypes.bfloat16
    hs = np.ascontiguousarray(np.asarray(inputs["hidden_states"], np.float32)[0])
    Wq = np.asarray(inputs["Wq"], np.float32)
    Wk = np.asarray(inputs["Wk"], np.float32)
    Wv = np.asarray(inputs["Wv"], np.float32)
    Wg = np.asarray(inputs["Wg"], np.float32)
    Wo = np.asarray(inputs["Wo"], np.float32)
    Ws1 = np.asarray(inputs["Ws1"], np.float32)
    Ws2 = np.asarray(inputs["Ws2"], np.float32)
    bs2 = np.asarray(inputs["bs2"], np.float32)
    gnw = np.asarray(inputs["g_norm_weight"], np.float32)

    hsT = hs.T  # [D, T]
    # hsb: [p, chunk, k, t] with d = k*128 + p
    hsb = np.ascontiguousarray(
        hsT.reshape(ND, 128, NB, 512).transpose(1, 2, 0, 3)).astype(BF)
    pos = np.arange(T, dtype=np.float64)
    inv = 1.0 / (ROPE_BASE ** (np.arange(0, DK, 2, dtype=np.float64) / DK))
    ang = pos[:, None] * inv[None, :]
    cos = np.cos(ang).T.astype(np.float32)       # [64, T]
    sin = np.sin(ang).T.astype(np.float32)
    cossin = np.concatenate([cos, sin], axis=0).astype(np.float32)
    triu = np.triu(np.ones((128, 128), np.float32)).astype(BF)
    masks = np.zeros((128, 4, 512), np.float32)
    p = np.arange(128)[:, None]
    r = np.arange(512)[None, :]
    for m in range(4):
        masks[:, m, :] = (128 * m + p <= r).astype(np.float32)
    ident = np.eye(128, dtype=np.float32)
    onesrow = np.ones((1, 128), np.float32)
    ones2k = np.ones((1, T), np.float32).astype(BF)
    onescol = np.ones((128, 1), np.float32)
    # woT: [p, kc, n] with hd = kc*128 + p; gnw folded in
    woT = (Wo.T * np.tile(gnw, H)[:, None]).astype(BF)
    woT = np.ascontiguousarray(woT.reshape(ND, 128, D).transpose(1, 0, 2))

    def wlay(w):  # [2048, 128] -> [p, k, c] bf16
        return np.ascontiguousarray(
            w.reshape(ND, 128, -1).transpose(1, 0, 2)).astype(BF)

    in_maps = []
    for core in range(N_CORES):
        sl = slice(core * 256, (core + 1) * 256)
        ssl = slice(core * 128, (core + 1) * 128)
        ws2e = np.concatenate([Ws2[ssl].T, bs2[None, ssl]], axis=0).astype(BF)

        m = {
            "hsb": hsb,
            "wq0": wlay(Wq[sl].T[:, 0:128] * SCALE),
            "wq1": wlay(Wq[sl].T[:, 128:256] * SCALE),
            "wk0": wlay(Wk[sl].T[:, 0:128]),
            "wk1": wlay(Wk[sl].T[:, 128:256]),
            "wg0": wlay(Wg[sl].T[:, 0:128]),
            "wg1": wlay(Wg[sl].T[:, 128:256]),
            "wv0": wlay(Wv[sl].T[:, 0:128]),
            "wv1": wlay(Wv[sl].T[:, 128:256]),
            "ws1": wlay(Ws1.T),
            "ws2e": ws2e,
            "onesrow_b": onesrow.astype(BF), "onescol_b": onescol.astype(BF),
            "onesrow_r": onesrow, "onescol_r": onescol,
            "ones2k": ones2k,
            "cossin": cossin,
            "triu": triu, "masks": masks.astype(BF), "ident": ident,
            "identb": ident.astype(BF),
            "woT": woT,
        }
        in_maps.append(m)
    return in_maps


_CACHE = {}


def kernel(**inputs):
    key = ("k", REPEAT, DEBUG)
    if key not in _CACHE:
        _CACHE[key] = build(repeat=REPEAT, debug=DEBUG)
    nc, dbg = _CACHE[key]
    in_maps = _host_inputs(inputs)
    res = bass_utils.run_bass_kernel_spmd(nc, in_maps, core_ids=list(range(N_CORES)))
    out = np.concatenate([res.results[c]["out"] for c in range(N_CORES)], axis=0)
    kernel.last_results = res
    return out.reshape(1, T, D).astype(np.float32)


# revision 20
# speedup vs baseline: 1.0718x; 1.0718x over previous
"""ABC attention (gated slot attention) on 8 TRN2 NeuronCores.

Sharding: 2 heads per core (16 heads / 8 cores). Per core:
  - projections q,k (RoPE, q pre-scaled), v, silu(gate), slot logits,
    all matmuls bf16 (2x stream rate vs f32r), moving dim 512
  - unnormalized softmax: RMS-norm downstream is scale-invariant, so
    softmax keeps only exp(ok*enz)*enz; enz applied in [m,t] layout
    (enzT) -> no per-row transposes/reductions/reciprocals
  - LINEAR-ATTENTION restructure: the unmasked (off-diagonal) prefix of
    the two quadratic stages collapses into rank-64 running states
      B[dk,m]  = sum_{j<prefix} k[j,dk] es[j,m]   (stage 1)
      CT[m,dv] = sum_{j<prefix} es[j,m] v[j,dv]   (stage 2)
    accumulated in a persistent PSUM bank; only the diagonal 512x512
    superblock runs the masked quadratic path (with per-128-block
    moving-dim shrink). Exact reassociation of the same unnormalized
    sums the quadratic version computes.
  - fused RMS-norm x gate epilogue (Rsqrt broadcast via PE)
  - AllToAll reshards o_g head-split -> T-split (1MB/core vs 8.4MB
    AllGather); full-size warm-up A2A mid-attention; A2A(h0) issued
    before h1's epilogue; o_proj accumulates h0 chunks first so it
    overlaps A2A(h1); PE filler matmuls hold DVFS clock during A2A(h0)
    and during the startup DMA window.
"""
import sys
if '/opt/trn_rl_repo' not in sys.path:
    sys.path.insert(0, '/opt/trn_rl_repo')
import numpy as np

import concourse.bacc as bacc
import concourse.mybir as mybir
import concourse.tile as tile
from concourse import bass_utils

F32 = mybir.dt.float32
F32R = mybir.dt.float32r
BF16 = mybir.dt.bfloat16
AF = mybir.ActivationFunctionType

H, DK, DV, M, T, D = 16, 128, 128, 64, 2048, 2048
EPS, CLAMP, ROPE_BASE = 1e-5, 32.0, 10000.0
N_CORES = 8
NT = T // 128        # 16
NB = T // 512        # 4 big row-chunks
ND = D // 128        # 16
SCALE = DK ** -0.5

REPEAT = 1
DEBUG = False
STARTUP_FILLERS = 44
A2A_FILLERS = 95


def build(repeat=1, debug=False):
    nc = bacc.Bacc(None, target_bir_lowering=False, debug=False, num_devices=N_CORES)

    din = {}
    for nm, shp, dt in [
        ("hsb", [128, NB, ND, 512], BF16),
        ("wq0", [128, ND, 128], BF16), ("wq1", [128, ND, 128], BF16),
        ("wk0", [128, ND, 128], BF16), ("wk1", [128, ND, 128], BF16),
        ("wg0", [128, ND, 128], BF16), ("wg1", [128, ND, 128], BF16),
        ("wvu", [128, ND, 272], BF16),
        ("ws2e", [17, 128], BF16),
        ("ones2k", [1, T], BF16),
        ("onesrow_b", [1, 128], BF16), ("onescol_b", [128, 1], BF16),
        ("onesrow_r", [1, 128], F32R), ("onescol_r", [128, 1], F32R),
        ("cossin", [128, T], F32),
        ("triu", [128, 128], BF16), ("ident", [128, 128], F32),
        ("identb", [128, 128], BF16),
        ("masks", [128, 4, 512], BF16),
        ("woT", [128, ND, D], BF16),
    ]:
        din[nm] = nc.dram_tensor(nm, shp, dt, kind="ExternalInput")
    out_d = nc.dram_tensor("out", [256, D], BF16, kind="ExternalOutput")

    dbg = {}
    if debug:
        for nm, shp, dt in [("qT", [256, T], BF16), ("kT", [256, T], BF16),
                            ("v", [128, NT * 256], BF16),
                            ("sg", [256, T], BF16), ("es", [128, NT * 128], BF16),
                            ("enzT", [128, T], F32), ("esT", [128, T], BF16),
                            ("u17", [17, T], BF16), ("qveT", [128, T], BF16),
                            ("ogT", [256, T], BF16)]:
            dbg[nm] = nc.dram_tensor("dbg_" + nm, shp, dt, kind="ExternalOutput")

    with tile.TileContext(nc) as tc:
        with tc.tile_pool(name="const", bufs=1) as cpool, \
             tc.tile_pool(name="big", bufs=1) as big:
            c = {}
            for nm in ("ws2e", "onesrow_b", "onescol_b", "onesrow_r",
                       "onescol_r", "triu", "ident", "identb"):
                tl = cpool.tile(list(din[nm].shape), din[nm].dtype, tag=nm, name=nm)
                nc.sync.dma_start(tl[:], din[nm].ap())
                c[nm] = tl

            st = {
                "bigpool": big,
                "qT": [big.tile([128, T], BF16, tag=f"qT{h}", name=f"qT{h}") for h in range(2)],
                "kT": [big.tile([128, T], BF16, tag=f"kT{h}", name=f"kT{h}") for h in range(2)],
                "k_t": [big.tile([128, NT, 128], BF16, tag=f"k_t{h}", name=f"k_t{h}") for h in range(2)],
                "sg": [big.tile([128, T], BF16, tag=f"sg{h}", name=f"sg{h}") for h in range(2)],
                "v": big.tile([128, NT, 256], BF16, tag="v", name="v"),
                "u17": big.tile([17, T], BF16, tag="u17", name="u17"),
                "es_t": big.tile([128, NT, 128], BF16, tag="es_t", name="es_t"),
                "esT": big.tile([128, T], BF16, tag="esT", name="esT"),
                "enzT": big.tile([128, T], F32, tag="enzT", name="enzT"),
                "B_sb": big.tile([128, 2, 64], BF16, tag="B_sb", name="B_sb"),
                "CT_sb": big.tile([128, 128], BF16, tag="CT_sb", name="CT_sb"),
            }
            for _ in range(repeat):
                _pass(nc, tc, din, c, st, out_d, dbg, debug)

    nc.compile()
    return nc, dbg


def _pass(nc, tc, din, c, st, out_d, dbg, debug):
    qT, kT, sg = st["qT"], st["kT"], st["sg"]
    v, u17, es_t, esT, enzT = st["v"], st["u17"], st["es_t"], st["esT"], st["enzT"]
    k_t, B_sb, CT_sb = st["k_t"], st["B_sb"], st["CT_sb"]

    def cpy(eng, dst, srcap):
        if eng is nc.scalar:
            nc.scalar.copy(dst, srcap)
        elif eng is nc.gpsimd:
            nc.gpsimd.tensor_copy(dst, srcap)
        else:
            nc.vector.tensor_copy(dst, srcap)

    # ================= PHASE 1: projections (4 x 512-col sweeps) =================
    with tc.tile_pool(name="p1w", bufs=1) as p1w, \
         tc.tile_pool(name="p1sb", bufs=2) as p1sb, \
         tc.tile_pool(name="p1hs", bufs=2) as p1hs:
        p1ps_cm = tc.tile_pool(name="p1ps", bufs=1, space="PSUM")
        p1ps = p1ps_cm.__enter__()
        # hs chunk 0 first (4 sub-tiles of 4 d-groups each), then weights:
        # the first matmul needs only hsq sub 0 + wq0, so compute starts
        # ~6us in instead of waiting for the full weight set
        engs = [nc.sync, nc.scalar, nc.gpsimd]

        # PE fillers: a memset tile needs no DMA, so the PE ramps its DVFS
        # clock during the initial weight/hs DMA window instead of idling
        warm_sb = p1sb.tile([128, 640], BF16, tag="warm_sb", bufs=1)
        nc.vector.memset(warm_sb[:], 0.0)
        fill_ps = p1ps.tile([128, 512], F32, tag="pa0", name="fill_ps")
        for i in range(STARTUP_FILLERS):
            nc.tensor.matmul(fill_ps[:], warm_sb[:, 0:128], warm_sb[:, 128:640],
                             start=True, stop=True)

        # weights on the scalar queue (clear of the 4MB hs stream on
        # sync/gpsimd) so the first matmul group unblocks ~5us in
        nc.gpsimd.dma_start(u17[16:17, :], din["ones2k"].ap())

        def hs_load_pair(tb0, tb1):
            # interleave the two chunks' sub-DMAs so sub0 of BOTH chunks
            # lands first (the d-loop consumes ti=0/ti=1 per d)
            subs = [[], []]
            for s in range(4):
                for ti, tb in ((0, tb0), (1, tb1)):
                    t = p1hs.tile([128, 4, 512], BF16, tag=f"hsq{s}",
                                  name=f"hsq{tb}_{s}")
                    eng = nc.sync if s % 2 == 0 else nc.gpsimd
                    eng.dma_start(t[:], din["hsb"].ap()[:, tb, 4 * s:4 * s + 4])
                    subs[ti].append(t)
            return subs

        hs_next = hs_load_pair(0, 1)
        wts = {}
        weng = {"wq0": nc.scalar, "wq1": nc.scalar, "wk0": nc.scalar,
                "wk1": nc.scalar, "wg0": nc.scalar, "wg1": nc.scalar,
                "wvu": nc.sync}
        for wn in ("wq0", "wq1", "wk0", "wk1", "wg0", "wg1", "wvu"):
            cw = din[wn].shape[2]
            wt = p1w.tile([128, ND, cw], BF16, tag=wn, name=wn)
            weng[wn].dma_start(wt[:], din[wn].ap())
            wts[wn] = wt
        cossin = p1w.tile([128, T], F32, tag="cossin", name="cossin")
        nc.scalar.dma_start(cossin[:], din["cossin"].ap())

        carries = []

        def mk_pt(pool, tags):
            state = {"i": 0}

            def pt(shape, name):
                t = pool.tile(shape, F32, tag=tags[state["i"] % len(tags)],
                              name=name)
                state["i"] += 1
                return t
            return pt

        def slot_chain(tb, pt, sbp):
            # staged: pse/esT -> pssq/es_t (one wide exp) -> csum chain ->
            # cs2 quad -> one wide recip -> transpose quad -> one enzT copy
            tsl = slice(tb * 512, (tb + 1) * 512)
            t0 = 4 * tb
            pse = pt([128, 512], f"psesT{tb}")
            nc.tensor.matmul(pse[:], c["ws2e"][:], u17[:, tsl], start=True, stop=True)
            nc.scalar.activation(esT[:, tsl], pse[:], AF.Exp)
            pssq = pt([128, 4, 128], f"ps_st{tb}")
            for i in range(4):
                nc.tensor.matmul(pssq[:, i, :],
                                 u17[:, (t0 + i) * 128:(t0 + i + 1) * 128],
                                 c["ws2e"][:], start=True, stop=True)
            nc.scalar.activation(
                es_t[:, t0:t0 + 4, :].rearrange("p a b -> p (a b)"),
                pssq[:].rearrange("p a b -> p (a b)"), AF.Exp)
            for i in range(4):
                ts = t0 + i
                if ts >= NT - 1:
                    break
                csum = pt([1, 128], f"csum{ts}")
                nc.tensor.matmul(csum[:], c["onescol_b"][:], es_t[:, ts, :],
                                 start=True, stop=(ts == 0))
                if ts > 0:
                    nc.tensor.matmul(csum[:], c["onesrow_b"][:, 0:1],
                                     carries[ts - 1][:], start=False, stop=True)
                cr = st["bigpool"].tile([1, 128], BF16, tag=f"carry{ts}",
                                        name=f"carry{ts}")
                nc.vector.tensor_copy(cr[:], csum[:])
                carries.append(cr)
            cs2q = pt([128, 4, 128], f"ps_cs{tb}")
            for i in range(4):
                ts = t0 + i
                nc.tensor.matmul(cs2q[:, i, :], c["triu"][:], es_t[:, ts, :],
                                 start=True, stop=(ts == 0))
                if ts > 0:
                    nc.tensor.matmul(cs2q[:, i, :], c["onesrow_b"][:],
                                     carries[ts - 1][:], start=False, stop=True)
            enz_sb = sbp.tile([128, 512], F32, tag="enz_sb", name=f"enz{tb}")
            nc.vector.reciprocal_approx_fast(
                enz_sb[:], cs2q[:].rearrange("p a b -> p (a b)"))
            etpq = pt([128, 4, 128], f"etp{tb}")
            for i in range(4):
                nc.tensor.transpose(etpq[:, i, :],
                                    enz_sb[:, i * 128:(i + 1) * 128],
                                    c["ident"][:])
            nc.scalar.copy(enzT[:, tsl], etpq[:].rearrange("p a b -> p (a b)"))
        st["slot_chain"] = slot_chain
        st["mk_pt"] = mk_pt

        QKG = {"wq0": (qT[0], "q"), "wq1": (qT[1], "q"),
               "wk0": (kT[0], "k"), "wk1": (kT[1], "k"),
               "wg0": (sg[0], "g"), "wg1": (sg[1], "g")}
        GROUPS = (("wq0", "wq1"), ("wk0", "wk1"), ("wg0", "wg1"))
        TAGSETS = (("pa0", "pa1", "pa2", "pa3"), ("pb0", "pb1", "pb2", "pb3"))

        def drain_qkg(wn, ti, ps):
            # drains: RoPE for q/k (q pre-scaled), silu for g
            dst, kind = QKG[wn]
            tsl = slice(ti * 512, (ti + 1) * 512)
            if kind == "g":
                nc.scalar.activation(dst[:, tsl], ps[:], AF.Silu)
            else:
                t1 = p1sb.tile([64, 512], F32, tag="ropet1")
                t2 = p1sb.tile([64, 512], F32, tag="ropet2")
                nc.vector.tensor_mul(t1[:], ps[0:64, :], cossin[0:64, tsl])
                nc.vector.tensor_mul(t2[:], ps[64:128, :], cossin[64:128, tsl])
                nc.vector.tensor_sub(dst[0:64, tsl], t1[:], t2[:])
                nc.vector.tensor_mul(t1[:], ps[64:128, :], cossin[0:64, tsl])
                nc.vector.tensor_mul(t2[:], ps[0:64, :], cossin[64:128, tsl])
                nc.vector.tensor_add(dst[64:128, tsl], t1[:], t2[:])

        def drain_vu(ts, accv_i):
            eng = nc.scalar if ts % 2 == 0 else nc.vector
            cpy(eng, v[:, ts, :], accv_i[:, 0:256])
            usb = p1sb.tile([128, 16], F32, tag="usb")
            cpy(eng, usb[:], accv_i[:, 256:272])
            utp = p1ps.tile([16, 128], F32, tag="pb2", name=f"utp{ts}")
            nc.tensor.transpose(utp[:], usb[:], c["ident"][:])
            cpy(eng, u17[0:16, ts * 128:(ts + 1) * 128], utp[:])

        # paired 512-chunks: each weight tile loads once per PAIR of chunks
        # (consecutive same-stationary matmuls run at stream rate)
        for pair in range(2):
            tbs = (2 * pair, 2 * pair + 1)
            hsq_p = hs_next
            if pair == 0:
                hs_next = hs_load_pair(2, 3)

            def hs_d(ti, d):
                return hsq_p[ti][d // 4][:, d % 4, :]

            for gi, grp in enumerate(GROUPS):
                tags = TAGSETS[(2 * pair + gi) % 2]
                acc = {}
                for wi, wn in enumerate(grp):
                    for ti in range(2):
                        acc[(wn, ti)] = p1ps.tile(
                            [128, 512], F32, tag=tags[2 * wi + ti],
                            name=f"acc_{wn}_{ti}_p{pair}")
                for d in range(ND):
                    for wn in grp:
                        for ti in range(2):
                            nc.tensor.matmul(acc[(wn, ti)][:], wts[wn][:, d, :],
                                             hs_d(ti, d),
                                             start=(d == 0), stop=(d == ND - 1))
                for wn in grp:
                    for ti in range(2):
                        drain_qkg(wn, tbs[ti], acc[(wn, ti)])
            # v+u rounds: 2 banks at a time (tags pa/pb rotate with groups: use
            # dedicated v tags on the tagset not used by the last group)
            vtags = TAGSETS[(2 * pair + 1) % 2][0:2]
            for ti in range(2):
                for half in range(2):
                    accv = [p1ps.tile([128, 272], F32, tag=vtags[i],
                                      name=f"accv{pair}{ti}{half}{i}")
                            for i in range(2)]
                    for d in range(ND):
                        for i in range(2):
                            nc.tensor.matmul(
                                accv[i][:],
                                hs_d(ti, d)[:, (2 * half + i) * 128:(2 * half + i + 1) * 128],
                                wts["wvu"][:, d, :],
                                start=(d == 0), stop=(d == ND - 1))
                    for i in range(2):
                        drain_vu(4 * tbs[ti] + 2 * half + i, accv[i])
            # slot chains: pair 0 inline; pair 1's both deferred to phase 2
            if pair == 0:
                for ti in range(2):
                    slot_chain(tbs[ti], mk_pt(p1ps, ("pb2", "pb3")), p1sb)
        p1ps_cm.__exit__(None, None, None)

    if debug:
        for h in range(2):
            nc.sync.dma_start(dbg["qT"].ap()[h * 128:(h + 1) * 128, :], qT[h][:])
            nc.sync.dma_start(dbg["kT"].ap()[h * 128:(h + 1) * 128, :], kT[h][:])
            nc.sync.dma_start(dbg["sg"].ap()[h * 128:(h + 1) * 128, :], sg[h][:])
        nc.sync.dma_start(dbg["v"].ap(), v[:].rearrange("p a b -> p (a b)"))
        nc.sync.dma_start(dbg["es"].ap(), es_t[:].rearrange("p a b -> p (a b)"))
        nc.sync.dma_start(dbg["enzT"].ap(), enzT[:])
        nc.sync.dma_start(dbg["esT"].ap(), esT[:])
        nc.sync.dma_start(dbg["u17"].ap(), u17[:])

    # ================= PHASE 2: attention =================
    with tc.tile_pool(name="p2sb", bufs=3) as p2sb, \
         tc.tile_pool(name="qveP", bufs=2) as qvep, \
         tc.tile_pool(name="dramp", bufs=1, space="DRAM") as dpool:
        ag_in, ag_out = [], []
        for h in range(2):
            ag_in.append(dpool.tile([N_CORES, 128, 256], BF16,
                                    tag=f"ag_in{h}", name=f"ag_in{h}"))
            ag_out.append(dpool.tile([N_CORES, 128, 256], BF16,
                                     tag=f"ag_out{h}", name=f"ag_out{h}"))
        warm_in = dpool.tile([N_CORES, 128, 256], BF16, tag="warm_in", name="warm_in")
        warm_out = dpool.tile([N_CORES, 128, 256], BF16, tag="warm_out", name="warm_out")
        wsrc = p2sb.tile([128, 256], BF16, tag="wsrc", bufs=1)
        nc.vector.memset(wsrc[:], 0.0)
        for s in range(N_CORES):
            nc.sync.dma_start(warm_in[s, :, :], wsrc[:])
        epsb = p2sb.tile([128, 1], F32, tag="epsb", bufs=1)
        nc.vector.memset(epsb[:], EPS)
        fsrc = p2sb.tile([128, 640], BF16, tag="fsrc", bufs=1)
        nc.vector.memset(fsrc[:], 0.0)
        tl = p2sb.tile(list(din["masks"].shape), BF16, tag="masks", name="masks", bufs=1)
        nc.sync.dma_start(tl[:], din["masks"].ap())
        c["masks"] = tl
        # o_proj weights: prefetch the full 8MB during attention
        wo_sb = p2sb.tile([128, ND, D], BF16, tag="wo_sb", name="wo_sb", bufs=1)
        nc.sync.dma_start(wo_sb[:], din["woT"].ap())

        with tc.tile_pool(name="ps_atw", bufs=1, space="PSUM") as ps_at, \
             tc.tile_pool(name="ps_ok", bufs=1, space="PSUM") as ps_ok, \
             tc.tile_pool(name="ps_ot", bufs=2, space="PSUM") as ps_ot, \
             tc.tile_pool(name="ps_pre", bufs=1, space="PSUM") as ps_pre:
            # PSUM: at(2) + w2(2, also epilogue aux) + okT(1) + oT(2) + pre(1)
            # persistent rank-64 prefix states, packed into one PSUM bank:
            #   B[h]  = pre[:, h*64:(h+1)*64]            [dk=128, m=64]
            #   CT[h] = pre[h*64:(h+1)*64, 128:256]      [m=64, dv=128]
            pre = ps_pre.tile([128, 512], F32, tag="pre", name="pre")

            def B_ps(h):
                return pre[:, h * 64:(h + 1) * 64]

            def CT_ps(h):
                return pre[h * 64:(h + 1) * 64, 128:256]

            def mk_pt2(pool):
                def pt(shape, name):
                    return pool.tile(shape, F32, tag="oT", name=name)
                return pt

            def gen_pre():
                # k in t-major layout for the B prefix updates: transpose kT
                # 128-blocks through the PE (4 at a time into one PSUM tile);
                # interleaved with stage1(0) as independent PE work
                for h in range(2):
                    for qd in range(4):
                        trp = ps_ot.tile([128, 4, 128], BF16, tag="oT",
                                         name=f"ktr{h}{qd}")
                        for i in range(4):
                            ts = 4 * qd + i
                            nc.tensor.transpose(trp[:, i, :],
                                                kT[h][:, ts * 128:(ts + 1) * 128],
                                                c["identb"][:])
                        eng = nc.scalar if (h + qd) % 2 == 0 else nc.vector
                        cpy(eng,
                            k_t[h][:, 4 * qd:4 * qd + 4, :].rearrange("p a b -> p (a b)"),
                            trp[:].rearrange("p a b -> p (a b)"))
                        yield
                st["slot_chain"](NB - 2, mk_pt2(ps_ot), p2sb)
                yield
                st["slot_chain"](NB - 1, mk_pt2(ps_ot), p2sb)
                yield

            def softmax(I, okT):
                # unnormalized: qveT = exp(okT * enzT) * enzT  (RMS-norm
                # downstream cancels the per-(t,h) softmax denominator)
                rsl = slice(I * 512, (I + 1) * 512)
                qveT = qvep.tile([128, 512], BF16, tag="qveT")
                okm = p2sb.tile([128, 512], F32, tag="okm")
                nc.vector.tensor_mul(okm[:], okT[:], enzT[:, rsl])
                eok = p2sb.tile([128, 512], F32, tag="eok")
                nc.scalar.activation(eok[:], okm[:], AF.Exp)
                nc.vector.tensor_mul(qveT[:], eok[:], enzT[:, rsl])
                return qveT

            def stage1_steps(I, res):
                njs0 = 4 * I
                okT = ps_ok.tile([128, 512], F32, tag="okT", name=f"okT{I}")
                at_sb = {}

                def at_pair(m):
                    j = njs0 + m
                    lo = 128 * m if I > 0 else 0
                    for h in range(2):
                        at = ps_at.tile([128, 512], F32, tag="at", bufs=2)
                        nc.tensor.matmul(at[:, lo:512],
                                         kT[h][:, j * 128:(j + 1) * 128],
                                         qT[h][:, I * 512 + lo:(I + 1) * 512],
                                         start=True, stop=True)
                        a_sb = p2sb.tile([128, 512], BF16, tag="at_sb")
                        if I == 0:
                            nc.vector.tensor_mul(a_sb[:], at[:], c["masks"][:, m, :])
                        else:
                            nc.vector.tensor_mul(
                                a_sb[:, lo:lo + 128], at[:, lo:lo + 128],
                                c["masks"][:, m, lo:lo + 128])
                            if lo + 128 < 512:
                                eng = nc.scalar if h == 0 else nc.vector
                                cpy(eng, a_sb[:, lo + 128:512],
                                    at[:, lo + 128:512])
                        at_sb[(m, h)] = (a_sb, lo)

                def ok_mm(m):
                    for h in range(2):
                        a_sb, lo = at_sb.pop((m, h))
                        nc.tensor.matmul(okT[h * 64:(h + 1) * 64, lo:512],
                                         es_t[:, njs0 + m, h * 64:(h + 1) * 64],
                                         a_sb[:, lo:512],
                                         start=(I == 0 and m == 0),
                                         stop=(m == 3),
                                         skip_group_check=True)

                # off-diagonal prefix: okT[h] = B^T q  (rank-64 state)
                if I > 0:
                    for h in range(2):
                        nc.tensor.matmul(okT[h * 64:(h + 1) * 64, :],
                                         B_sb[:, h, :], qT[h][:, I * 512:(I + 1) * 512],
                                         start=True, stop=False,
                                         skip_group_check=True)
                at_pair(0)
                yield
                at_pair(1)
                yield
                for m in range(4):
                    ok_mm(m)
                    if m + 2 < 4:
                        at_pair(m + 2)
                    yield
                # softmax as early as possible: stage2(I) consumes qveT first
                # thing next iteration
                res["qv"] = softmax(I, okT)
                yield
                # B prefix update with this superblock's own k/es, then
                # refresh the bf16 stationary copy for stage1(I+1)
                if I < NB - 1:
                    for h in range(2):
                        for mi in range(4):
                            ts = njs0 + mi
                            nc.tensor.matmul(B_ps(h), k_t[h][:, ts, :],
                                             es_t[:, ts, h * 64:(h + 1) * 64],
                                             start=(I == 0 and mi == 0),
                                             stop=(mi == 3),
                                             skip_group_check=True)
                    yield
                    for h in range(2):
                        eng = nc.scalar if h == 0 else nc.vector
                        cpy(eng, B_sb[:, h, :], B_ps(h))
                if I == 2:
                    # full-size warm-up A2A: brings the collective path to
                    # steady-state so the real transfers at the end run warm
                    nc.gpsimd.collective_compute(
                        "AllToAll", mybir.AluOpType.bypass,
                        replica_groups=[list(range(N_CORES))],
                        ins=[warm_in[:].opt()], outs=[warm_out[:].opt()])
                yield

            def stage2_steps(I, qveT):
                rsl = slice(I * 512, (I + 1) * 512)
                njs0 = 4 * I
                oT = [ps_ot.tile([128, 512], F32, tag="oT", name=f"oT{i}") for i in range(2)]
                w2_sb = {}

                def w2_pair(m, hs=(0, 1)):
                    j = njs0 + m
                    lo = 128 * m if I > 0 else 0
                    for h in hs:
                        w2 = ps_at.tile([128, 512], F32, tag="w2", bufs=2)
                        nc.tensor.matmul(w2[:, lo:512],
                                         esT[h * 64:(h + 1) * 64, j * 128:(j + 1) * 128],
                                         qveT[h * 64:(h + 1) * 64, lo:512],
                                         start=True, stop=True)
                        wsb = p2sb.tile([128, 512], BF16, tag="at_sb")
                        if I == 0:
                            nc.vector.tensor_mul(wsb[:], w2[:], c["masks"][:, m, :])
                        else:
                            nc.vector.tensor_mul(
                                wsb[:, lo:lo + 128], w2[:, lo:lo + 128],
                                c["masks"][:, m, lo:lo + 128])
                            if lo + 128 < 512:
                                eng = nc.scalar if h == 0 else nc.vector
                                cpy(eng, wsb[:, lo + 128:512],
                                    w2[:, lo + 128:512])
                        w2_sb[(m, h)] = (wsb, lo)

                def o_mm(m, hs=(0, 1)):
                    for h in hs:
                        wsb, lo = w2_sb.pop((m, h))
                        nc.tensor.matmul(oT[h][:, lo:512],
                                         v[:, njs0 + m, h * 128:(h + 1) * 128],
                                         wsb[:, lo:512],
                                         start=(I == 0 and m == 0),
                                         stop=(m == 3),
                                         skip_group_check=True)

                def o_off(h):
                    nc.tensor.matmul(oT[h][:],
                                     CT_sb[h * 64:(h + 1) * 64, :],
                                     qveT[h * 64:(h + 1) * 64, :],
                                     start=True, stop=False,
                                     skip_group_check=True)

                def epilogue(h):
                    sq = p2sb.tile([128, 512], F32R, tag="sq")
                    nc.scalar.activation(sq[:], oT[h][:], AF.Square)
                    ssq = ps_at.tile([1, 512], F32, tag="w2", bufs=2)
                    nc.tensor.matmul(ssq[:], c["onescol_r"][:], sq[:], start=True, stop=True)
                    ssq_sb = p2sb.tile([1, 512], F32R, tag="ssq_sb")
                    with nc.allow_low_precision(reason="f32r bitcast for broadcast"):
                        nc.scalar.copy(ssq_sb[:], ssq[:])
                    rb = ps_at.tile([128, 512], F32, tag="w2", bufs=2)
                    nc.tensor.matmul(rb[:], c["onesrow_r"][:], ssq_sb[:], start=True, stop=True)
                    yield
                    rms = p2sb.tile([128, 512], F32, tag="rms")
                    nc.scalar.activation(rms[:], rb[:], AF.Sqrt, scale=1.0 / DV,
                                         bias=epsb[:])
                    rinv = p2sb.tile([128, 512], F32, tag="rinv")
                    nc.vector.reciprocal_approx_fast(rinv[:], rms[:])
                    t1 = p2sb.tile([128, 512], F32, tag="ept1")
                    nc.vector.tensor_mul(t1[:], oT[h][:], sg[h][:, rsl])
                    ogt = p2sb.tile([128, 512], BF16, tag="ogt")
                    nc.vector.tensor_mul(ogt[:], t1[:], rinv[:])
                    if debug:
                        nc.gpsimd.dma_start(dbg["ogT"].ap()[h * 128:(h + 1) * 128, rsl], ogt[:])
                    nc.sync.dma_start(ag_in[h][2 * I, :, :], ogt[:, 0:256])
                    nc.sync.dma_start(ag_in[h][2 * I + 1, :, :], ogt[:, 256:512])
                    if I == NB - 1:
                        # kick this head's A2A the moment its last slots land;
                        # h0's transfer overlaps h1's compute + o_proj(h0)
                        nc.gpsimd.collective_compute(
                            "AllToAll", mybir.AluOpType.bypass,
                            replica_groups=[list(range(N_CORES))],
                            ins=[ag_in[h][:].opt()], outs=[ag_out[h][:].opt()])
                    yield

                if I == NB - 1:
                    # last chunk: finish h0's whole path (accumulate, epilogue,
                    # A2A) before touching h1, so A2A(h0) fires ~8us earlier
                    # and rides a freshly-warmed collective path
                    nc.gpsimd.collective_compute(
                        "AllToAll", mybir.AluOpType.bypass,
                        replica_groups=[list(range(N_CORES))],
                        ins=[warm_in[:].opt()], outs=[warm_out[:].opt()])
                    for h in range(2):
                        o_off(h)
                        w2_pair(0, hs=(h,))
                        yield
                        w2_pair(1, hs=(h,))
                        yield
                        for m in range(4):
                            o_mm(m, hs=(h,))
                            if m + 2 < 4:
                                w2_pair(m + 2, hs=(h,))
                            yield
                        yield from epilogue(h)
                    return
                # off-diagonal prefix: oT[h] = CT^T qve  (rank-64 state)
                if I > 0:
                    for h in range(2):
                        o_off(h)
                w2_pair(0)
                yield
                w2_pair(1)
                yield
                for m in range(4):
                    o_mm(m)
                    if m + 2 < 4:
                        w2_pair(m + 2)
                    yield
                # CT prefix update + bf16 refresh
                if I < NB - 1:
                    for h in range(2):
                        for mi in range(4):
                            ts = njs0 + mi
                            nc.tensor.matmul(CT_ps(h),
                                             es_t[:, ts, h * 64:(h + 1) * 64],
                                             v[:, ts, h * 128:(h + 1) * 128],
                                             start=(I == 0 and mi == 0),
                                             stop=(mi == 3),
                                             skip_group_check=True)
                    yield
                    for h in range(2):
                        eng = nc.scalar if h == 0 else nc.vector
                        cpy(eng, CT_sb[h * 64:(h + 1) * 64, :], CT_ps(h))
                    yield
                # epilogue: o_g = o * rsqrt(mean o^2 + eps) * sg -> bf16 -> a2a_in
                for h in range(2):
                    yield from epilogue(h)

            def interleave(*gs):
                gens = list(gs)
                while gens:
                    for g in list(gens):
                        try:
                            next(g)
                        except StopIteration:
                            gens.remove(g)

            # software pipeline: stage1(I) and stage2(I-1) emit interleaved so
            # the PE queue always holds ready work from the other stream while
            # one stream waits on its vector/scalar producers
            res = {}
            interleave(stage1_steps(0, res), gen_pre())
            qv_prev = res["qv"]
            if debug:
                nc.sync.dma_start(dbg["qveT"].ap()[:, 0:512], qv_prev[:])
            for I in range(1, NB):
                interleave(stage1_steps(I, res), stage2_steps(I - 1, qv_prev))
                qv_prev = res["qv"]
                if debug:
                    nc.sync.dma_start(dbg["qveT"].ap()[:, I * 512:(I + 1) * 512], qv_prev[:])
            def filler_gen(n, per=6):
                # zero-data matmuls: keep the PE clock ramped through the
                # final stage2 + A2A window without burning switching power
                i = 0
                while i < n:
                    fps = ps_ok.tile([128, 512], F32, tag="okT", name=f"fill{i}")
                    for _ in range(min(per, n - i)):
                        nc.tensor.matmul(fps[:], fsrc[:, 0:128],
                                         fsrc[:, 128:640], start=True, stop=True)
                    i += per
                    yield

            interleave(stage2_steps(NB - 1, qv_prev), filler_gen(A2A_FILLERS))

        # ================= PHASE 3: per-head A2A + o_proj =================
        og = {}
        dma_engs3 = [nc.sync, nc.scalar, nc.gpsimd]
        for h in range(2):
            for s in range(N_CORES):
                ot = p2sb.tile([128, 256], BF16, tag=f"og{h}{s}",
                               name=f"og{h}{s}", bufs=1)
                dma_engs3[(h * N_CORES + s) % 3].dma_start(ot[:], ag_out[h][s, :, :])
                og[(h, s)] = ot
        p3ps_cm = tc.tile_pool(name="p3ps", bufs=1, space="PSUM")
        p3ps = p3ps_cm.__enter__()
        pso = [[p3ps.tile([128, 512], F32, tag=f"pso{th}{ns}",
                          name=f"pso{th}{ns}", bufs=1) for ns in range(4)]
               for th in range(2)]
        # o_proj: h0 chunks first so the h0 half overlaps A2A(h1)
        order = [(0, s) for s in range(N_CORES)] + [(1, s) for s in range(N_CORES)]
        for n, (h, s) in enumerate(order):
            kc = 2 * s + h
            for th in range(2):
                for ns in range(4):
                    nc.tensor.matmul(pso[th][ns][:],
                                     og[(h, s)][:, th * 128:(th + 1) * 128],
                                     wo_sb[:, kc, ns * 512:(ns + 1) * 512],
                                     start=(n == 0), stop=(n == 15))
        dma_engs = [nc.sync, nc.scalar, nc.gpsimd]
        for th in range(2):
            for ns in range(4):
                osb = p2sb.tile([128, 512], BF16, tag="osb")
                eng = nc.scalar if ns % 2 == 0 else nc.vector
                cpy(eng, osb[:], pso[th][ns][:])
                dma_engs[(4 * th + ns) % 3].dma_start(
                    out_d.ap()[th * 128:(th + 1) * 128, ns * 512:(ns + 1) * 512],
                    osb[:])
        p3ps_cm.__exit__(None, None, None)


# ======================= host side =======================

def _host_inputs(inputs):
    import ml_dtypes
    BF = ml_dtypes.bfloat16
    hs = np.ascontiguousarray(np.asarray(inputs["hidden_states"], np.float32)[0])
    Wq = np.asarray(inputs["Wq"], np.float32)
    Wk = np.asarray(inputs["Wk"], np.float32)
    Wv = np.asarray(inputs["Wv"], np.float32)
    Wg = np.asarray(inputs["Wg"], np.float32)
    Wo = np.asarray(inputs["Wo"], np.float32)
    Ws1 = np.asarray(inputs["Ws1"], np.float32)
    Ws2 = np.asarray(inputs["Ws2"], np.float32)
    bs2 = np.asarray(inputs["bs2"], np.float32)
    gnw = np.asarray(inputs["g_norm_weight"], np.float32)

    hsT = hs.T  # [D, T]
    # hsb: [p, chunk, k, t] with d = k*128 + p
    hsb = np.ascontiguousarray(
        hsT.reshape(ND, 128, NB, 512).transpose(1, 2, 0, 3)).astype(BF)
    pos = np.arange(T, dtype=np.float64)
    inv = 1.0 / (ROPE_BASE ** (np.arange(0, DK, 2, dtype=np.float64) / DK))
    ang = pos[:, None] * inv[None, :]
    cos = np.cos(ang).T.astype(np.float32)       # [64, T]
    sin = np.sin(ang).T.astype(np.float32)
    cossin = np.concatenate([cos, sin], axis=0).astype(np.float32)
    triu = np.triu(np.ones((128, 128), np.float32)).astype(BF)
    masks = np.zeros((128, 4, 512), np.float32)
    p = np.arange(128)[:, None]
    r = np.arange(512)[None, :]
    for m in range(4):
        masks[:, m, :] = (128 * m + p <= r).astype(np.float32)
    ident = np.eye(128, dtype=np.float32)
    onesrow = np.ones((1, 128), np.float32)
    ones2k = np.ones((1, T), np.float32).astype(BF)
    onescol = np.ones((128, 1), np.float32)
    # woT: [p, kc, n] with hd = kc*128 + p; gnw folded in
    woT = (Wo.T * np.tile(gnw, H)[:, None]).astype(BF)
    woT = np.ascontiguousarray(woT.reshape(ND, 128, D).transpose(1, 0, 2))

    def wlay(w):  # [2048, 128] -> [p, k, c] bf16
        return np.ascontiguousarray(
            w.reshape(ND, 128, -1).transpose(1, 0, 2)).astype(BF)

    in_maps = []
    for core in range(N_CORES):
        sl = slice(core * 256, (core + 1) * 256)
        ssl = slice(core * 128, (core + 1) * 128)
        ws2e = np.concatenate([Ws2[ssl].T, bs2[None, ssl]], axis=0).astype(BF)
        wvu = np.concatenate([Wv[sl].T, Ws1.T], axis=1)  # [2048, 272]
        m = {
            "hsb": hsb,
            "wq0": wlay(Wq[sl].T[:, 0:128] * SCALE),
            "wq1": wlay(Wq[sl].T[:, 128:256] * SCALE),
            "wk0": wlay(Wk[sl].T[:, 0:128]),
            "wk1": wlay(Wk[sl].T[:, 128:256]),
            "wg0": wlay(Wg[sl].T[:, 0:128]),
            "wg1": wlay(Wg[sl].T[:, 128:256]),
            "wvu": wlay(wvu),
            "ws2e": ws2e,
            "onesrow_b": onesrow.astype(BF), "onescol_b": onescol.astype(BF),
            "onesrow_r": onesrow, "onescol_r": onescol,
            "ones2k": ones2k,
            "cossin": cossin,
            "triu": triu, "masks": masks.astype(BF), "ident": ident,
            "identb": ident.astype(BF),
            "woT": woT,
        }
        in_maps.append(m)
    return in_maps


_CACHE = {}


def kernel(**inputs):
    key = ("k", REPEAT, DEBUG)
    if key not in _CACHE:
        _CACHE[key] = build(repeat=REPEAT, debug=DEBUG)
    nc, dbg = _CACHE[key]
    in_maps = _host_inputs(inputs)
    res = bass_utils.run_bass_kernel_spmd(nc, in_maps, core_ids=list(range(N_CORES)))
    out = np.concatenate([res.results[c]["out"] for c in range(N_CORES)], axis=0)
    kernel.last_results = res
    return out.reshape(1, T, D).astype(np.float32)
